# revision 15
# baseline (speedup 1.0000x reference)
"""Trainium2 Bass kernel for nn_NoiseGenerator.

Reference (per batch lane j of 1024, N=24000 samples):
    lp1 = one_pole_lowpass(noise, f_lp)          # y[n] = a*y[n-1] + (1-a)*x[n]
    hp  = lp1 - one_pole_lowpass(lp1, f_hp)
    out = hp * env * gain,  env = (1-exp(-t/(attack+eps))) * exp(-t/(decay+eps))

Restructured (validated ~5e-6 rel err vs the f32 reference):
    S1[n] = alpha*S1[n-1] + u[n]                 (alpha = f_lp; raw noise input)
    T[n]  = beta*T[n-1] + S1[n]                  (beta = f_hp)
    Pn    = (1-beta)*T - S1                      (= -(lp1-lp2)/(1-alpha))
    out   = Pn * genvn,  genvn = -g1*(E1 - E2),  g1 = gain*(1-alpha)
    E1[n] = exp(-n*qd/SR), E2[n] = exp(-n*(qa+qd)/SR)   (geometric in n)

Sharding: voice-parallel, 1024 lanes -> 8 cores x 128 lanes (one partition per
lane), time on the free dim in 1024-col chunks.

Engine split (DVE and GPSIMD share an SBUF port lock, so GPSIMD is unused):
  PE  : (a) broadcast u across partitions via exact bf16-split K=2 matmul
        (u = uh + ul, weights 1.0, fp32 PSUM accumulate — error ~8e-6)
        (b) the whole envelope as diagonal-weight matmuls with bf16 splits:
        genvn = diag(-g1*e1[q])@PowD + diag(g1*e2[q])@PowAD, each factor split
        hi/lo so the result carries ~1.5e-5 error; accumulated in fp32 PSUM.
        E[n0+d] = E[n0]*r^d power tables and diag matrices come from the host.
  DVE : the two IIR scans (native tensor_tensor_scan), Pn, out
  ACT/GPSIMD/idle.
Host folds gain*(1-alpha) and all signs into the diag tables and transposes the
gathered [128, 24000] per-core outputs into the full [24000, 1024] result.
"""

import os
import sys

import numpy as np

for _p in ("/opt/trn_rl_repo",):
    if _p not in sys.path and os.path.isdir(_p):
        sys.path.insert(0, _p)

N = 24000
B = 1024
NCORES = 8
LANES = 128
C = 1024  # chunk columns (2 PSUM banks per [128, C] f32 tile)
SR = 48000.0
EPS = 1e-4

_CHUNKS = []
_s = 0
while _s < N:
    _CHUNKS.append((_s, min(C, N - _s)))
    _s += C
NCHUNK = len(_CHUNKS)

_compiled = None


def _build_program():
    import concourse.bacc as bacc
    import concourse.mybir as mybir
    import concourse.tile as tile

    f32 = mybir.dt.float32
    bf16 = mybir.dt.bfloat16
    Alu = mybir.AluOpType
    Act = mybir.ActivationFunctionType

    nc = bacc.Bacc(
        "TRN2", target_bir_lowering=False, debug=False, num_devices=NCORES
    )

    u2_dram = nc.dram_tensor("u2", [2, N], bf16, kind="ExternalInput")
    acol_dram = nc.dram_tensor("acol", [LANES, 1], f32, kind="ExternalInput")
    bcol_dram = nc.dram_tensor("bcol", [LANES, 1], f32, kind="ExternalInput")
    ombcol_dram = nc.dram_tensor("ombcol", [LANES, 1], f32, kind="ExternalInput")
    # bf16 hi/lo splits of the envelope power tables [r^d for d in 0..C)
    pdh_dram = nc.dram_tensor("pdh", [LANES, C], bf16, kind="ExternalInput")
    pdl_dram = nc.dram_tensor("pdl", [LANES, C], bf16, kind="ExternalInput")
    pah_dram = nc.dram_tensor("pah", [LANES, C], bf16, kind="ExternalInput")
    pal_dram = nc.dram_tensor("pal", [LANES, C], bf16, kind="ExternalInput")
    # per-chunk diagonal weight matrices (already scaled/negated on host)
    d1h_dram = nc.dram_tensor("d1h", [NCHUNK, LANES, LANES], bf16, kind="ExternalInput")
    d1l_dram = nc.dram_tensor("d1l", [NCHUNK, LANES, LANES], bf16, kind="ExternalInput")
    d2h_dram = nc.dram_tensor("d2h", [NCHUNK, LANES, LANES], bf16, kind="ExternalInput")
    d2l_dram = nc.dram_tensor("d2l", [NCHUNK, LANES, LANES], bf16, kind="ExternalInput")
    out_dram = nc.dram_tensor("out", [LANES, N], f32, kind="ExternalOutput")

    with tile.TileContext(nc) as tc:
        with (
            tc.tile_pool(name="const", bufs=1) as constp,
            tc.tile_pool(name="work", bufs=4) as work,
            tc.tile_pool(name="chain", bufs=4) as chain,
            tc.tile_pool(name="psum_u", bufs=2, space="PSUM") as psum_u,
            tc.tile_pool(name="psum_e", bufs=2, space="PSUM") as psum_e,
        ):
            ones2 = constp.tile([2, LANES], bf16)
            nc.vector.memset(ones2[:], 1.0)
            acol = constp.tile([LANES, 1], f32)
            nc.sync.dma_start(acol[:], acol_dram[:])
            bcol = constp.tile([LANES, 1], f32)
            nc.sync.dma_start(bcol[:], bcol_dram[:])
            ombcol = constp.tile([LANES, 1], f32)
            nc.sync.dma_start(ombcol[:], ombcol_dram[:])
            pdh = constp.tile([LANES, C], bf16)
            nc.sync.dma_start(pdh[:], pdh_dram[:])
            pdl = constp.tile([LANES, C], bf16)
            nc.sync.dma_start(pdl[:], pdl_dram[:])
            pah = constp.tile([LANES, C], bf16)
            nc.sync.dma_start(pah[:], pah_dram[:])
            pal = constp.tile([LANES, C], bf16)
            nc.sync.dma_start(pal[:], pal_dram[:])

            zt = constp.tile([LANES, C], f32)
            nc.vector.memset(zt[:], 0.0)
            abc = constp.tile([LANES, C], f32)
            nc.vector.tensor_scalar(abc[:], zt[:], acol[:], None, Alu.add)
            bbc = constp.tile([LANES, C], f32)
            nc.vector.tensor_scalar(bbc[:], zt[:], bcol[:], None, Alu.add)

            prev_s1 = None
            prev_t = None
            for q, (n0, cw) in enumerate(_CHUNKS):
                uq = work.tile([2, C], bf16, tag="uq")
                nc.sync.dma_start(uq[:, :cw], u2_dram[:, n0 : n0 + cw])
                d1h = work.tile([LANES, LANES], bf16, tag="d1h")
                nc.sync.dma_start(d1h[:], d1h_dram[q])
                d1l = work.tile([LANES, LANES], bf16, tag="d1l")
                nc.sync.dma_start(d1l[:], d1l_dram[q])
                d2h = work.tile([LANES, LANES], bf16, tag="d2h")
                nc.sync.dma_start(d2h[:], d2h_dram[q])
                d2l = work.tile([LANES, LANES], bf16, tag="d2l")
                nc.sync.dma_start(d2l[:], d2l_dram[q])

                u1 = psum_u.tile([LANES, C], f32, tag="u1")
                for s0 in range(0, cw, 512):
                    sw = min(512, cw - s0)
                    nc.tensor.matmul(
                        u1[:, s0 : s0 + sw], ones2[:], uq[:, s0 : s0 + sw]
                    )

                # genvn = d1h@PDh + d1h@PDl + d1l@PDh + d2h@PAh + d2h@PAl + d2l@PAh
                genv = psum_e.tile([LANES, C], f32, tag="genv")
                terms = [
                    (d1h, pdh), (d1h, pdl), (d1l, pdh),
                    (d2h, pah), (d2h, pal), (d2l, pah),
                ]
                # term-major: one LDWEIGHTS per diag matrix per chunk
                for ti, (dw, pw) in enumerate(terms):
                    for s0 in range(0, cw, 512):
                        sw = min(512, cw - s0)
                        nc.tensor.matmul(
                            genv[:, s0 : s0 + sw],
                            dw[:],
                            pw[:, s0 : s0 + sw],
                            start=(ti == 0),
                            stop=(ti == len(terms) - 1),
                        )

                s1 = chain.tile([LANES, C], f32, tag="s1")
                nc.vector.tensor_tensor_scan(
                    s1[:, :cw],
                    abc[:, :cw],
                    u1[:, :cw],
                    0.0 if q == 0 else prev_s1[:, C - 1 : C],
                    Alu.mult,
                    Alu.add,
                )
                t_ = chain.tile([LANES, C], f32, tag="t_")
                nc.vector.tensor_tensor_scan(
                    t_[:, :cw],
                    bbc[:, :cw],
                    s1[:, :cw],
                    0.0 if q == 0 else prev_t[:, C - 1 : C],
                    Alu.mult,
                    Alu.add,
                )
                # Pn = (1-beta)*T - S1 = -P on DVE
                pn = work.tile([LANES, C], f32, tag="pn")
                nc.vector.scalar_tensor_tensor(
                    pn[:, :cw], t_[:, :cw], ombcol[:], s1[:, :cw],
                    Alu.mult, Alu.subtract,
                )
                # evacuate genv PSUM->SBUF on the idle ScalarE so GPSIMD
                # (which has no PSUM access) can do the final multiply while
                # the DVE keeps scanning the next chunks
                genv_s = work.tile([LANES, C], f32, tag="genv_s")
                nc.scalar.activation(genv_s[:, :cw], genv[:, :cw], Act.Copy)
                # out = Pn * genvn = P * genv on GPSIMD
                oc = work.tile([LANES, C], f32, tag="oc")
                nc.gpsimd.tensor_tensor(
                    oc[:, :cw], pn[:, :cw], genv_s[:, :cw], Alu.mult
                )
                nc.sync.dma_start(out_dram[:, n0 : n0 + cw], oc[:, :cw])

                prev_s1 = s1
                prev_t = t_

    nc.compile()
    return nc


def _bfsplit(x32):
    """Exact-ish bf16 hi/lo split of a float32 array."""
    import ml_dtypes

    bf16 = ml_dtypes.bfloat16
    h = x32.astype(np.float32).astype(bf16)
    l = (x32.astype(np.float32) - h.astype(np.float32)).astype(bf16)
    return h, l


def _host_prep(parameters, noise):
    """Per-core input maps. All derived scalars computed in float64."""
    p = np.asarray(parameters, dtype=np.float64)
    u = np.asarray(noise, dtype=np.float32).reshape(N)
    attack, decay, f_lp, f_hp, gain = p

    uh, ul = _bfsplit(u)
    u2 = np.stack([uh, ul], axis=0)  # [2, N] bf16

    alpha = f_lp
    beta = f_hp
    g1 = gain * (1.0 - alpha)
    qd = 1.0 / (decay + EPS)
    qad = qd + 1.0 / (attack + EPS)

    d = np.arange(C, dtype=np.float64)
    starts = np.array([c0 for c0, _ in _CHUNKS], dtype=np.float64)
    eye = np.eye(LANES, dtype=np.float32)

    in_maps = []
    for c in range(NCORES):
        ln = slice(c * LANES, (c + 1) * LANES)
        powd = np.exp(-qd[ln, None] * d[None, :] / SR).astype(np.float32)
        powad = np.exp(-qad[ln, None] * d[None, :] / SR).astype(np.float32)
        pdh, pdl = _bfsplit(powd)
        pah, pal = _bfsplit(powad)
        # genvn = -g1*(E1 - E2): e1 diag negative, e2 diag positive
        e1 = (-g1[ln, None] * np.exp(-qd[ln, None] * starts[None, :] / SR)).astype(
            np.float32
        )  # [128, NCHUNK]
        e2 = (g1[ln, None] * np.exp(-qad[ln, None] * starts[None, :] / SR)).astype(
            np.float32
        )
        e1h, e1l = _bfsplit(e1)
        e2h, e2l = _bfsplit(e2)
        # diag tensors [NCHUNK, 128, 128]: lhsT layout diag(col) works either way
        d1h = eye[None, :, :] * e1h.astype(np.float32).T[:, None, :]
        d1l = eye[None, :, :] * e1l.astype(np.float32).T[:, None, :]
        d2h = eye[None, :, :] * e2h.astype(np.float32).T[:, None, :]
        d2l = eye[None, :, :] * e2l.astype(np.float32).T[:, None, :]
        in_maps.append(
            {
                "u2": u2,
                "acol": alpha[ln, None].astype(np.float32),
                "bcol": beta[ln, None].astype(np.float32),
                "ombcol": (1.0 - beta)[ln, None].astype(np.float32),
                "pdh": pdh, "pdl": pdl, "pah": pah, "pal": pal,
                "d1h": _tobf(d1h), "d1l": _tobf(d1l),
                "d2h": _tobf(d2h), "d2l": _tobf(d2l),
            }
        )
    return in_maps


def _tobf(x):
    import ml_dtypes

    return x.astype(ml_dtypes.bfloat16)


def kernel(parameters, noise):
    global _compiled
    from concourse.bass_utils import run_bass_kernel_spmd

    if _compiled is None:
        _compiled = _build_program()
    nc = _compiled

    in_maps = _host_prep(parameters, noise)
    res = run_bass_kernel_spmd(nc, in_maps, core_ids=list(range(NCORES)))
    kernel.last_results = res

    out = np.empty((N, B), dtype=np.float32)
    for c in range(NCORES):
        out[:, c * LANES : (c + 1) * LANES] = res.results[c]["out"].T
    return out
